# revision 38
# baseline (speedup 1.0000x reference)
"""Multi-head self-attention (B=4, N=2048, C=512, H=8) on 8 trn2 NeuronCores.

Sharding: core = 2*b + g  (b = batch, g = head-half).  Each core handles one
batch element and 4 heads (channel slice of 256), computes its partial output
projection y^T = W_p[:, slice] @ out[slice], and the host sums the two
partials per batch element (fp32) and adds b_proj.

All matmul operands are fp16 (PSUM accumulation fp32).  The exp stream on
the ACT engine (128 x [128,1024] tiles, ~1.1us each) is the throughput
floor; emission keeps ACT saturated from ~15us and the PE continuously fed:

  q/k head-pair 0, column chunk 0 only               [kqps pool, PSUM right]
  unit (h0,p2=0): scores+exp, fillers: kq_j0 chunk 1 + v tiles
  unit (h0,p2=1): scores+exp, fillers: kq_j1 chunks  [all in kqps tiles]
  kqps releases -> otps opens (right)
  units 2..7: scores+exp + AV of unit u-2 (lag-2, deep pT ring)
  tail: AV(u6)+chain, AV(u7)+chain, then 2-pass projection: pass1 j=0
  (ACT Copy to SBUF - same ACT table as Exp), pass2 j=1 (DVE add), fp16
  output on 2 DMA queues.
"""

import numpy as np

import concourse.bacc as bacc
import concourse.bass as bass
import concourse.mybir as mybir
import concourse.tile as tile
from concourse.bass_utils import run_bass_kernel_spmd

B, N, C, H, HD = 4, 2048, 512, 8, 64
HPC, CS = 4, 256  # heads per core, channels per core
SCALE = HD ** -0.5
F16 = mybir.dt.float16
F32 = mybir.dt.float32
NCORES = 8
MT = N // 128  # 16 key tiles

_NC = None


def _build():
    nc = bacc.Bacc("TRN2", target_bir_lowering=False, debug=False,
                   num_devices=NCORES)
    xT_d = nc.dram_tensor("xT", [C, N], F16, kind="ExternalInput")
    wqT_d = nc.dram_tensor("wqT", [C, CS], F16, kind="ExternalInput")
    wkT_d = nc.dram_tensor("wkT", [C, CS], F16, kind="ExternalInput")
    wvT_d = nc.dram_tensor("wvT", [C, CS], F16, kind="ExternalInput")
    wpT_d = nc.dram_tensor("wpT", [CS, C], F16, kind="ExternalInput")
    bq_d = nc.dram_tensor("bq", [128, 2], F32, kind="ExternalInput")
    bk_d = nc.dram_tensor("bk", [128, 2], F32, kind="ExternalInput")
    bvb_d = nc.dram_tensor("bvb", [128, CS], F16, kind="ExternalInput")
    ones_col_d = nc.dram_tensor("ones_col", [128, HPC], F16,
                                kind="ExternalInput")
    yT_d = nc.dram_tensor("yT", [C, N], F16, kind="ExternalOutput")

    with tile.TileContext(nc) as tc:
        with (
            tc.tile_pool(name="const", bufs=1) as const,
            tc.tile_pool(name="big", bufs=1) as big,
            tc.tile_pool(name="xt", bufs=1) as xtp,
            tc.tile_pool(name="pt", bufs=40) as ptp,
            tc.tile_pool(name="recip", bufs=2) as rcp,
            tc.tile_pool(name="rbc", bufs=2) as rbcp,
            tc.tile_pool(name="ysb", bufs=4) as ysbp,
        ):
            # ---- input DMAs: x as 8 half-tiles on 2 queues (first halves
            # of all 4 ct tiles first, so the kq c0 chunks start early) ----
            xt = [xtp.tile([128, N], F16, tag=f"x{ct}", name=f"x{ct}")
                  for ct in range(4)]
            for ct in range(4):
                eng = nc.sync if ct % 2 == 0 else nc.scalar
                eng.dma_start(out=xt[ct][:], in_=xT_d[bass.ts(ct, 128), :])
            wq_t, wk_t, wv_t = [], [], []
            for lst, nm, src in ((wq_t, "wq", wqT_d), (wk_t, "wk", wkT_d),
                                 (wv_t, "wv", wvT_d)):
                for ct in range(4):
                    t = const.tile([128, CS], F16, tag=f"{nm}{ct}",
                                   name=f"{nm}{ct}")
                    nc.gpsimd.dma_start(out=t[:], in_=src[bass.ts(ct, 128), :])
                    lst.append(t)
            bq_sb = const.tile([128, 2], F32, tag="bq", name="bq")
            nc.gpsimd.dma_start(out=bq_sb[:], in_=bq_d[:])
            bk_sb = const.tile([128, 2], F32, tag="bk", name="bk")
            nc.gpsimd.dma_start(out=bk_sb[:], in_=bk_d[:])
            # v bias pre-broadcast host-side so v tiles need no bias matmul
            bvb = const.tile([128, CS], F16, tag="bvb", name="bvb")
            nc.gpsimd.dma_start(out=bvb[:], in_=bvb_d[:])
            wp_t = []
            for j in range(2):
                t = const.tile([128, C], F16, tag=f"wp{j}", name=f"wp{j}")
                nc.gpsimd.dma_start(out=t[:], in_=wpT_d[bass.ts(j, 128), :])
                wp_t.append(t)

            # ---- persistent activations ---------------------------------
            # q^T/k^T as separate tiles per column chunk c so a chunk can
            # be produced while earlier chunks are already being read
            # (same-tile write-after-read races on hardware).
            qT_c = [[big.tile([128, 1024], F16, tag=f"qT{j}_{c}",
                              name=f"qT{j}_{c}") for c in range(2)]
                    for j in range(2)]
            kT_c = [[big.tile([128, 1024], F16, tag=f"kT{j}_{c}",
                              name=f"kT{j}_{c}") for c in range(2)]
                    for j in range(2)]
            v1m = [big.tile([128, HPC, HD + 1], F16, tag=f"v1m_{m}",
                            name=f"v1m_{m}") for m in range(MT)]
            for m in range(MT):
                nc.gpsimd.dma_start(
                    out=v1m[m][:, :, HD:HD + 1],
                    in_=ones_col_d[:, :].rearrange("p (h o) -> p h o", o=1),
                )
            oT_pair = [big.tile([128, N], F16, tag=f"oT{j}", name=f"oT{j}")
                       for j in range(2)]

            # kq chunk: one [128,1024] column chunk of q^T or k^T for head
            # pair j, accumulated over the 4 input-channel tiles + bias.
            def kq_chunk(pool, w_t, b_sb, dst, j, c):
                ps = pool.tile([128, 1024], F32, tag="kq", name="kq")
                for ct in range(4):
                    for hf in range(2):
                        nc.tensor.matmul(
                            ps[:, bass.ts(hf, 512)],
                            lhsT=w_t[ct][:, bass.ts(j, 128)],
                            rhs=xt[ct][:, c * 1024 + hf * 512:
                                       c * 1024 + hf * 512 + 512],
                            start=(ct == 0), stop=(ct == 3),
                        )
                nc.vector.tensor_scalar_add(
                    dst[j][c][:], ps[:], b_sb[:, j:j + 1])

            # v tile m: [128 tok, 256 ch], bias added in the PSUM->SBUF
            # copy (bvb is the host-pre-broadcast v bias).
            def v_tile(pool, m):
                vp = pool.tile([128, 1024], F32, tag="kq", name="kq")
                for ct in range(4):
                    nc.tensor.matmul(
                        vp[:, 0:CS],
                        lhsT=xt[ct][:, bass.ts(m, 128)],
                        rhs=wv_t[ct][:],
                        start=(ct == 0), stop=(ct == 3),
                    )
                nc.vector.tensor_add(v1m[m][:, :, 0:HD], vp[:, 0:CS],
                                     bvb[:])

            def scores_exp(h, p2, m):
                j, hh = h // 2, h % 2
                psl = slice(hh * 64, hh * 64 + 64)
                sT = stps.tile([128, 1024], F32, tag="sT", name="sT")
                for hf in range(2):
                    nc.tensor.matmul(
                        sT[:, bass.ts(hf, 512)],
                        lhsT=kT_c[j][m // 8][psl, bass.ts(m % 8, 128)],
                        rhs=qT_c[j][p2][psl, bass.ts(hf, 512)],
                        start=True, stop=True,
                    )
                pT = ptp.tile([128, 1024], F16, tag="pT", name="pT")
                nc.scalar.activation(
                    out=pT[:], in_=sT[:],
                    func=mybir.ActivationFunctionType.Exp,
                    scale=SCALE,
                )
                return pT

            def av(oT, h, m, pT, first, last):
                for hf in range(2):
                    nc.tensor.matmul(
                        oT[:, bass.ts(hf, 512)],
                        lhsT=v1m[m][:, h, :],
                        rhs=pT[:, bass.ts(hf, 512)],
                        start=first, stop=last,
                    )

            def chain(oT, h, p2):
                j, hh = h // 2, h % 2
                psl = slice(hh * 64, hh * 64 + 64)
                den = rcp.tile([1, 1024], F32, tag="den", name="den")
                nc.vector.tensor_copy(den[:], oT[HD:HD + 1, :])
                rc = rcp.tile([1, 1024], F32, tag="rc", name="rc")
                nc.vector.reciprocal_approx_fast(rc[:], den[:])
                bc = rbcp.tile([64, 1024], F32, tag="bc", name="bc")
                nc.gpsimd.partition_broadcast(bc[:], rc[:])
                nc.vector.tensor_mul(
                    oT_pair[j][psl, bass.ts(p2, 1024)], oT[0:HD, :], bc[:])

            units = [(h, p2) for h in range(HPC) for p2 in range(2)]
            pT_ring = {}

            kqps = tc.alloc_tile_pool(name="kqps", bufs=2, space="PSUM",
                                      side="right")
            # PE warmup: dummy matmuls on scratch data while the input
            # DMAs are in flight, so the PE p-state is fully ramped when
            # the real qkv matmuls start (cold PE runs at ~half clock for
            # the first ~3us of activity).
            scratch = const.tile([128, 512], F16, tag="scr", name="scr")
            nc.gpsimd.memset(scratch[:], 0.0)
            wu = kqps.tile([128, 1024], F32, tag="kq", name="kq")
            for _ in range(18):
                nc.tensor.matmul(wu[:, 0:512], lhsT=scratch[:, 0:128],
                                 rhs=scratch[:], start=True, stop=True)
            kq_chunk(kqps, wq_t, bq_sb, qT_c, 0, 0)
            kq_chunk(kqps, wk_t, bk_sb, kT_c, 0, 0)
            kq_chunk(kqps, wq_t, bq_sb, qT_c, 0, 1)
            kq_chunk(kqps, wk_t, bk_sb, kT_c, 0, 1)

            stps = tc.alloc_tile_pool(name="stps", bufs=2, space="PSUM",
                                      side="left")
            # ---- unit 0 (h0,p2=0): fillers = v tiles --------------------
            vq = list(range(MT))  # v tiles still to emit
            pT_ring[0] = []
            for m in range(MT):
                pT_ring[0].append(scores_exp(0, 0, m))
                if vq:
                    v_tile(kqps, vq.pop(0))
            # ---- unit 1 (h0,p2=1): fillers = kq_j1 bursts + v rest ------
            kq1 = [(wq_t, bq_sb, qT_c, 1, 0), (wk_t, bk_sb, kT_c, 1, 0),
                   (wq_t, bq_sb, qT_c, 1, 1), (wk_t, bk_sb, kT_c, 1, 1)]
            pT_ring[1] = []
            for m in range(MT):
                pT_ring[1].append(scores_exp(0, 1, m))
                if m % 4 == 1:
                    w_t, b_sb, dst, j, c = kq1[m // 4]
                    kq_chunk(kqps, w_t, b_sb, dst, j, c)
                elif vq:
                    v_tile(kqps, vq.pop(0))
            kqps.release()

            otps = tc.alloc_tile_pool(name="otps", bufs=2, space="PSUM",
                                      side="right")
            # ---- units 2..7: scores/exp + AV of unit u-2 (lag) ----------
            for u in range(2, 8):
                h, p2 = units[u]
                ph, pp2 = units[u - 2]
                oT_prev = otps.tile([HD + 1, 1024], F32, tag="oT", name="oT")
                pT_ring[u] = []
                for m in range(MT):
                    pT_ring[u].append(scores_exp(h, p2, m))
                    av(oT_prev, ph, m, pT_ring[u - 2][m],
                       first=(m == 0), last=(m == MT - 1))
                chain(oT_prev, ph, pp2)
            stps.release()

            # ---- tail: AVs for units 6,7, chains, 2-pass projection -----
            for u in (6, 7):
                h, p2 = units[u]
                oT = otps.tile([HD + 1, 1024], F32, tag="oT", name="oT")
                for m in range(MT):
                    av(oT, h, m, pT_ring[u][m],
                       first=(m == 0), last=(m == MT - 1))
                chain(oT, h, p2)
            otps.release()

            yps = tc.alloc_tile_pool(name="yps", bufs=4, space="PSUM",
                                     side="left")
            # projection: accumulate both j-halves in PSUM per (jj, c2)
            # chunk (waves of 4 tiles), single DVE cast, DMA on 2 queues.
            # c2-major so chunks gated only by the earlier chains go first
            # (c2 0/1 need chain u6, c2 2/3 need chain u7).
            for c2 in range(4):
                yws = []
                for jj in range(4):
                    yp = yps.tile([128, 512], F32, tag="yp", name="yp")
                    yws.append(yp)
                    nc.tensor.matmul(
                        yp[:],
                        lhsT=wp_t[0][:, bass.ts(jj, 128)],
                        rhs=oT_pair[0][:, bass.ts(c2, 512)],
                        start=True, stop=False,
                    )
                for jj in range(4):
                    nc.tensor.matmul(
                        yws[jj][:],
                        lhsT=wp_t[1][:, bass.ts(jj, 128)],
                        rhs=oT_pair[1][:, bass.ts(c2, 512)],
                        start=False, stop=True,
                    )
                    ys = ysbp.tile([128, 512], F16, tag="ys", name="ys")
                    nc.vector.tensor_copy(ys[:], yws[jj][:])
                    eng = nc.sync if jj % 2 == 0 else nc.scalar
                    eng.dma_start(
                        out=yT_d[bass.ts(jj, 128), bass.ts(c2, 512)],
                        in_=ys[:])
            yps.release()

    nc.compile()
    return nc


def get_nc():
    global _NC
    if _NC is None:
        _NC = _build()
    return _NC


def shard_inputs(x, w_qkv, b_qkv, w_proj, b_proj):
    x = np.asarray(x, dtype=np.float32)
    w_qkv = np.asarray(w_qkv, dtype=np.float32)
    b_qkv = np.asarray(b_qkv, dtype=np.float32)
    w_proj = np.asarray(w_proj, dtype=np.float32)
    ones_col = np.ones((128, HPC), np.float16)
    in_maps = []
    for core in range(NCORES):
        b, g = core // 2, core % 2
        sl = slice(g * CS, (g + 1) * CS)
        in_maps.append({
            "xT": np.ascontiguousarray(x[b].T).astype(np.float16),
            "wqT": np.ascontiguousarray(w_qkv[sl, :].T).astype(np.float16),
            "wkT": np.ascontiguousarray(w_qkv[C:][sl, :].T).astype(np.float16),
            "wvT": np.ascontiguousarray(
                w_qkv[2 * C:][sl, :].T).astype(np.float16),
            "wpT": np.ascontiguousarray(w_proj[:, sl].T).astype(np.float16),
            "bq": np.ascontiguousarray(b_qkv[sl].reshape(2, 128).T),
            "bk": np.ascontiguousarray(b_qkv[C:][sl].reshape(2, 128).T),
            "bvb": np.tile(b_qkv[2 * C:][sl].reshape(1, CS),
                           (128, 1)).astype(np.float16),
            "ones_col": ones_col,
        })
    return in_maps


def gather_output(results, b_proj):
    b_proj = np.asarray(b_proj, dtype=np.float32)
    out = np.empty((B, N, C), np.float32)
    for b in range(B):
        yT = (results[2 * b]["yT"].astype(np.float32)
              + results[2 * b + 1]["yT"].astype(np.float32))
        out[b] = yT.T + b_proj[None, :]
    return out


def kernel(x, w_qkv, b_qkv, w_proj, b_proj):
    nc = get_nc()
    in_maps = shard_inputs(x, w_qkv, b_qkv, w_proj, b_proj)
    # run twice: the first (cold) execution warms DMA paths; the second
    # run's timing/result is representative of steady state.
    run_bass_kernel_spmd(nc, in_maps, core_ids=list(range(NCORES)))
    res = run_bass_kernel_spmd(nc, in_maps, core_ids=list(range(NCORES)))
    return gather_output(res.results, b_proj)


# revision 40
# speedup vs baseline: 1.1161x; 1.1161x over previous
"""Multi-head self-attention (B=4, N=2048, C=512, H=8) on 8 trn2 NeuronCores.

Sharding: core = 2*b + g  (b = batch, g = head-half).  Each core handles one
batch element and 4 heads (channel slice of 256), computes its partial output
projection y^T = W_p[:, slice] @ out[slice], and the host sums the two
partials per batch element (fp32) and adds b_proj.

All matmul operands are fp16 (PSUM accumulation fp32).  The exp stream on
the ACT engine (128 x [128,1024] tiles, ~1.1us each) is the throughput
floor; emission keeps ACT saturated from ~15us and the PE continuously fed:

  q/k head-pair 0, column chunk 0 only               [kqps pool, PSUM right]
  unit (h0,p2=0): scores+exp, fillers: kq_j0 chunk 1 + v tiles
  unit (h0,p2=1): scores+exp, fillers: kq_j1 chunks  [all in kqps tiles]
  kqps releases -> otps opens (right)
  units 2..7: scores+exp + AV of unit u-2 (lag-2, deep pT ring)
  tail: AV(u6)+chain, AV(u7)+chain, then 2-pass projection: pass1 j=0
  (ACT Copy to SBUF - same ACT table as Exp), pass2 j=1 (DVE add), fp16
  output on 2 DMA queues.
"""

import numpy as np

import concourse.bacc as bacc
import concourse.bass as bass
import concourse.mybir as mybir
import concourse.tile as tile
from concourse.bass_utils import run_bass_kernel_spmd

B, N, C, H, HD = 4, 2048, 512, 8, 64
HPC, CS = 4, 256  # heads per core, channels per core
SCALE = HD ** -0.5
F16 = mybir.dt.float16
F32 = mybir.dt.float32
NCORES = 8
MT = N // 128  # 16 key tiles

_NC = None


def _build():
    nc = bacc.Bacc("TRN2", target_bir_lowering=False, debug=False,
                   num_devices=NCORES)
    xT_d = nc.dram_tensor("xT", [C, N], F16, kind="ExternalInput")
    wqT_d = nc.dram_tensor("wqT", [C, CS], F16, kind="ExternalInput")
    wkT_d = nc.dram_tensor("wkT", [C, CS], F16, kind="ExternalInput")
    wvT_d = nc.dram_tensor("wvT", [C, CS], F16, kind="ExternalInput")
    wpT_d = nc.dram_tensor("wpT", [CS, C], F16, kind="ExternalInput")
    bq_d = nc.dram_tensor("bq", [128, 2], F32, kind="ExternalInput")
    bk_d = nc.dram_tensor("bk", [128, 2], F32, kind="ExternalInput")
    bvb_d = nc.dram_tensor("bvb", [128, CS], F16, kind="ExternalInput")
    ones_col_d = nc.dram_tensor("ones_col", [128, HPC], F16,
                                kind="ExternalInput")
    yT_d = nc.dram_tensor("yT", [C, N], F16, kind="ExternalOutput")

    with tile.TileContext(nc) as tc:
        with (
            tc.tile_pool(name="const", bufs=1) as const,
            tc.tile_pool(name="big", bufs=1) as big,
            tc.tile_pool(name="xt", bufs=1) as xtp,
            tc.tile_pool(name="pt", bufs=40) as ptp,
            tc.tile_pool(name="recip", bufs=2) as rcp,
            tc.tile_pool(name="rbc", bufs=2) as rbcp,
            tc.tile_pool(name="ysb", bufs=4) as ysbp,
        ):
            # ---- input DMAs: x as 8 half-tiles on 2 queues (first halves
            # of all 4 ct tiles first, so the kq c0 chunks start early) ----
            # scratch memset is the FIRST gpsimd instruction so the PE
            # warmup below is not stuck behind the weight-DMA issue queue
            scratch = const.tile([128, 512], F16, tag="scr", name="scr")
            nc.gpsimd.memset(scratch[:], 0.0)
            xt = [xtp.tile([128, N], F16, tag=f"x{ct}", name=f"x{ct}")
                  for ct in range(4)]
            for ct in range(4):
                eng = nc.sync if ct % 2 == 0 else nc.scalar
                eng.dma_start(out=xt[ct][:], in_=xT_d[bass.ts(ct, 128), :])
            wq_t, wk_t, wv_t = [], [], []
            for lst, nm, src in ((wq_t, "wq", wqT_d), (wk_t, "wk", wkT_d),
                                 (wv_t, "wv", wvT_d)):
                for ct in range(4):
                    t = const.tile([128, CS], F16, tag=f"{nm}{ct}",
                                   name=f"{nm}{ct}")
                    nc.gpsimd.dma_start(out=t[:], in_=src[bass.ts(ct, 128), :])
                    lst.append(t)
            bq_sb = const.tile([128, 2], F32, tag="bq", name="bq")
            nc.gpsimd.dma_start(out=bq_sb[:], in_=bq_d[:])
            bk_sb = const.tile([128, 2], F32, tag="bk", name="bk")
            nc.gpsimd.dma_start(out=bk_sb[:], in_=bk_d[:])
            # v bias pre-broadcast host-side so v tiles need no bias matmul
            bvb = const.tile([128, CS], F16, tag="bvb", name="bvb")
            nc.gpsimd.dma_start(out=bvb[:], in_=bvb_d[:])
            wp_t = []
            for j in range(2):
                t = const.tile([128, C], F16, tag=f"wp{j}", name=f"wp{j}")
                nc.gpsimd.dma_start(out=t[:], in_=wpT_d[bass.ts(j, 128), :])
                wp_t.append(t)

            # ---- persistent activations ---------------------------------
            # q^T/k^T as separate tiles per column chunk c so a chunk can
            # be produced while earlier chunks are already being read
            # (same-tile write-after-read races on hardware).
            qT_c = [[big.tile([128, 1024], F16, tag=f"qT{j}_{c}",
                              name=f"qT{j}_{c}") for c in range(2)]
                    for j in range(2)]
            kT_c = [[big.tile([128, 1024], F16, tag=f"kT{j}_{c}",
                              name=f"kT{j}_{c}") for c in range(2)]
                    for j in range(2)]
            v1m = [big.tile([128, HPC, HD + 1], F16, tag=f"v1m_{m}",
                            name=f"v1m_{m}") for m in range(MT)]
            for m in range(MT):
                nc.gpsimd.dma_start(
                    out=v1m[m][:, :, HD:HD + 1],
                    in_=ones_col_d[:, :].rearrange("p (h o) -> p h o", o=1),
                )
            oT_pair = [big.tile([128, N], F16, tag=f"oT{j}", name=f"oT{j}")
                       for j in range(2)]

            # kq chunk: one [128,1024] column chunk of q^T or k^T for head
            # pair j, accumulated over the 4 input-channel tiles + bias.
            def kq_chunk(pool, w_t, b_sb, dst, j, c):
                ps = pool.tile([128, 1024], F32, tag="kq", name="kq")
                for ct in range(4):
                    for hf in range(2):
                        nc.tensor.matmul(
                            ps[:, bass.ts(hf, 512)],
                            lhsT=w_t[ct][:, bass.ts(j, 128)],
                            rhs=xt[ct][:, c * 1024 + hf * 512:
                                       c * 1024 + hf * 512 + 512],
                            start=(ct == 0), stop=(ct == 3),
                        )
                nc.vector.tensor_scalar_add(
                    dst[j][c][:], ps[:], b_sb[:, j:j + 1])

            # v tile m: [128 tok, 256 ch], bias added in the PSUM->SBUF
            # copy (bvb is the host-pre-broadcast v bias).
            def v_tile(pool, m):
                vp = pool.tile([128, 1024], F32, tag="kq", name="kq")
                for ct in range(4):
                    nc.tensor.matmul(
                        vp[:, 0:CS],
                        lhsT=xt[ct][:, bass.ts(m, 128)],
                        rhs=wv_t[ct][:],
                        start=(ct == 0), stop=(ct == 3),
                    )
                nc.vector.tensor_add(v1m[m][:, :, 0:HD], vp[:, 0:CS],
                                     bvb[:])

            def scores_exp(h, p2, m):
                j, hh = h // 2, h % 2
                psl = slice(hh * 64, hh * 64 + 64)
                sT = stps.tile([128, 1024], F32, tag="sT", name="sT")
                for hf in range(2):
                    nc.tensor.matmul(
                        sT[:, bass.ts(hf, 512)],
                        lhsT=kT_c[j][m // 8][psl, bass.ts(m % 8, 128)],
                        rhs=qT_c[j][p2][psl, bass.ts(hf, 512)],
                        start=True, stop=True,
                    )
                pT = ptp.tile([128, 1024], F16, tag="pT", name="pT")
                nc.scalar.activation(
                    out=pT[:], in_=sT[:],
                    func=mybir.ActivationFunctionType.Exp,
                    scale=SCALE,
                )
                return pT

            def av(oT, h, m, pT, first, last):
                for hf in range(2):
                    nc.tensor.matmul(
                        oT[:, bass.ts(hf, 512)],
                        lhsT=v1m[m][:, h, :],
                        rhs=pT[:, bass.ts(hf, 512)],
                        start=first, stop=last,
                    )

            def chain(oT, h, p2):
                j, hh = h // 2, h % 2
                psl = slice(hh * 64, hh * 64 + 64)
                den = rcp.tile([1, 1024], F32, tag="den", name="den")
                nc.vector.tensor_copy(den[:], oT[HD:HD + 1, :])
                rc = rcp.tile([1, 1024], F32, tag="rc", name="rc")
                nc.vector.reciprocal_approx_fast(rc[:], den[:])
                bc = rbcp.tile([64, 1024], F32, tag="bc", name="bc")
                nc.gpsimd.partition_broadcast(bc[:], rc[:])
                nc.vector.tensor_mul(
                    oT_pair[j][psl, bass.ts(p2, 1024)], oT[0:HD, :], bc[:])

            units = [(h, p2) for h in range(HPC) for p2 in range(2)]
            pT_ring = {}

            kqps = tc.alloc_tile_pool(name="kqps", bufs=2, space="PSUM",
                                      side="right")
            # PE warmup: dummy matmuls on scratch data while the input
            # DMAs are in flight, so the PE p-state is fully ramped when
            # the real qkv matmuls start (cold PE runs at ~half clock for
            # the first ~3us of activity).
            wu = kqps.tile([128, 1024], F32, tag="kq", name="kq")
            for i in range(12):
                nc.tensor.matmul(wu[:, bass.ts(i % 2, 512)],
                                 lhsT=scratch[:, 0:128],
                                 rhs=scratch[:], start=True, stop=True)
            kq_chunk(kqps, wq_t, bq_sb, qT_c, 0, 0)
            kq_chunk(kqps, wk_t, bk_sb, kT_c, 0, 0)
            kq_chunk(kqps, wq_t, bq_sb, qT_c, 0, 1)
            kq_chunk(kqps, wk_t, bk_sb, kT_c, 0, 1)

            stps = tc.alloc_tile_pool(name="stps", bufs=2, space="PSUM",
                                      side="left")
            # ---- unit 0 (h0,p2=0): fillers = v tiles --------------------
            vq = list(range(MT))  # v tiles still to emit
            pT_ring[0] = []
            for m in range(MT):
                pT_ring[0].append(scores_exp(0, 0, m))
                if vq:
                    v_tile(kqps, vq.pop(0))
            # ---- unit 1 (h0,p2=1): fillers = kq_j1 bursts + v rest ------
            kq1 = [(wq_t, bq_sb, qT_c, 1, 0), (wk_t, bk_sb, kT_c, 1, 0),
                   (wq_t, bq_sb, qT_c, 1, 1), (wk_t, bk_sb, kT_c, 1, 1)]
            pT_ring[1] = []
            for m in range(MT):
                pT_ring[1].append(scores_exp(0, 1, m))
                if m % 4 == 1:
                    w_t, b_sb, dst, j, c = kq1[m // 4]
                    kq_chunk(kqps, w_t, b_sb, dst, j, c)
                elif vq:
                    v_tile(kqps, vq.pop(0))
            kqps.release()

            otps = tc.alloc_tile_pool(name="otps", bufs=2, space="PSUM",
                                      side="right")
            # ---- units 2..7: scores/exp + AV of unit u-2 (lag) ----------
            for u in range(2, 8):
                h, p2 = units[u]
                ph, pp2 = units[u - 2]
                oT_prev = otps.tile([HD + 1, 1024], F32, tag="oT", name="oT")
                pT_ring[u] = []
                for m in range(MT):
                    pT_ring[u].append(scores_exp(h, p2, m))
                    av(oT_prev, ph, m, pT_ring[u - 2][m],
                       first=(m == 0), last=(m == MT - 1))
                chain(oT_prev, ph, pp2)
            stps.release()

            # ---- tail: AVs for units 6,7, chains, 2-pass projection -----
            for u in (6, 7):
                h, p2 = units[u]
                oT = otps.tile([HD + 1, 1024], F32, tag="oT", name="oT")
                for m in range(MT):
                    av(oT, h, m, pT_ring[u][m],
                       first=(m == 0), last=(m == MT - 1))
                chain(oT, h, p2)
            otps.release()

            yps = tc.alloc_tile_pool(name="yps", bufs=4, space="PSUM",
                                     side="left")
            # projection: accumulate both j-halves in PSUM per (jj, c2)
            # chunk (waves of 4 tiles), single DVE cast, DMA on 2 queues.
            # c2-major so chunks gated only by the earlier chains go first
            # (c2 0/1 need chain u6, c2 2/3 need chain u7).
            for c2 in range(4):
                yws = []
                for jj in range(4):
                    yp = yps.tile([128, 512], F32, tag="yp", name="yp")
                    yws.append(yp)
                    nc.tensor.matmul(
                        yp[:],
                        lhsT=wp_t[0][:, bass.ts(jj, 128)],
                        rhs=oT_pair[0][:, bass.ts(c2, 512)],
                        start=True, stop=False,
                    )
                for jj in range(4):
                    nc.tensor.matmul(
                        yws[jj][:],
                        lhsT=wp_t[1][:, bass.ts(jj, 128)],
                        rhs=oT_pair[1][:, bass.ts(c2, 512)],
                        start=False, stop=True,
                    )
                    ys = ysbp.tile([128, 512], F16, tag="ys", name="ys")
                    nc.vector.tensor_copy(ys[:], yws[jj][:])
                    eng = nc.sync if jj % 2 == 0 else nc.scalar
                    eng.dma_start(
                        out=yT_d[bass.ts(jj, 128), bass.ts(c2, 512)],
                        in_=ys[:])
            yps.release()

    nc.compile()
    return nc


def get_nc():
    global _NC
    if _NC is None:
        _NC = _build()
    return _NC


def shard_inputs(x, w_qkv, b_qkv, w_proj, b_proj):
    x = np.asarray(x, dtype=np.float32)
    w_qkv = np.asarray(w_qkv, dtype=np.float32)
    b_qkv = np.asarray(b_qkv, dtype=np.float32)
    w_proj = np.asarray(w_proj, dtype=np.float32)
    ones_col = np.ones((128, HPC), np.float16)
    in_maps = []
    for core in range(NCORES):
        b, g = core // 2, core % 2
        sl = slice(g * CS, (g + 1) * CS)
        in_maps.append({
            "xT": np.ascontiguousarray(x[b].T).astype(np.float16),
            "wqT": np.ascontiguousarray(w_qkv[sl, :].T).astype(np.float16),
            "wkT": np.ascontiguousarray(w_qkv[C:][sl, :].T).astype(np.float16),
            "wvT": np.ascontiguousarray(
                w_qkv[2 * C:][sl, :].T).astype(np.float16),
            "wpT": np.ascontiguousarray(w_proj[:, sl].T).astype(np.float16),
            "bq": np.ascontiguousarray(b_qkv[sl].reshape(2, 128).T),
            "bk": np.ascontiguousarray(b_qkv[C:][sl].reshape(2, 128).T),
            "bvb": np.tile(b_qkv[2 * C:][sl].reshape(1, CS),
                           (128, 1)).astype(np.float16),
            "ones_col": ones_col,
        })
    return in_maps


def gather_output(results, b_proj):
    b_proj = np.asarray(b_proj, dtype=np.float32)
    out = np.empty((B, N, C), np.float32)
    for b in range(B):
        yT = (results[2 * b]["yT"].astype(np.float32)
              + results[2 * b + 1]["yT"].astype(np.float32))
        out[b] = yT.T + b_proj[None, :]
    return out


def kernel(x, w_qkv, b_qkv, w_proj, b_proj):
    nc = get_nc()
    in_maps = shard_inputs(x, w_qkv, b_qkv, w_proj, b_proj)
    # run twice: the first (cold) execution warms DMA paths; the second
    # run's timing/result is representative of steady state.
    run_bass_kernel_spmd(nc, in_maps, core_ids=list(range(NCORES)))
    res = run_bass_kernel_spmd(nc, in_maps, core_ids=list(range(NCORES)))
    return gather_output(res.results, b_proj)
